# revision 59
# baseline (speedup 1.0000x reference)
"""Trainium2 Bass kernel for per-position FC decoder stack.

out[b, o3, p] = W3[p] @ (W2[p] @ (W1[p] @ glf[b] + b1[p]) + b2[p]) + b3[p]

Shapes: glf [32, 512, 1], W1 [2048, 32, 512], W2 [2048, 8, 32], W3 [2048, 3, 8].

All layers are linear, so the whole per-position affine map is folded on
the PE.  A32[p] = W3[p] @ W2[p] ([3, 32]) is computed from W2's natural
chunk layout (lhsT = [(p,o2) x o1]) against a zero-padded block-diag
W3^T.  Then m2T chunks (= (A32 @ W1)^T = [i, (o3, p)]) come from the
operand-swap trick: W1's natural 128-row chunks are the stationary
operand and a block-diag band of A32 (96 cols per 32 positions) is the
moving operand, so the 128 MiB W1 is never transposed.  Stage 2 applies
m2T to glf^T plus bias rows and lands directly in the output layout.

W1 is streamed as bf16 via gpsimd (SWDGE) cast-DMA -- halving the HBM
bytes -- and is fully SBUF-resident, so the stream never stalls on
buffer reuse.  bf16 moving operands run the PE at 1 cycle/row at any
width, which makes the narrow 96-col bands viable.

Host-side prep is layout-only (transposes / reindexing / zero-padding
of the small tensors, no arithmetic): w2c, bd3h, glfTh, b1r, b2r, b3rh.

Pipeline: the block-diag scatter runs two 32-position periods ahead;
stage 2 trails one period so its inputs are never copy-latency bound;
the final period is split into two 16-position halves so the tail after
the last W1 byte is minimal; outputs for periods 0-6 batch into one
contiguous-run DMA.  TimelineSim: 32415 ns (baseline 66342).

Sharding: positions (2048) split across 8 cores; glf replicated.
"""

import sys

if "/opt/trn_rl_repo" not in sys.path:
    sys.path.insert(0, "/opt/trn_rl_repo")

import numpy as np

# Problem constants (hardcoded per contest contract)
P_FULL = 2048
NCORES = 8
PP = P_FULL // NCORES  # 256 positions per core
B = 32
I = 512
O1 = 32
O2 = 8
O3 = 3
NT = 16    # W1 tiles of [128, 2048] (16 positions each)
NTT = 8    # tt-blocks of 32 positions
BW = 124   # BDA band stride: 96 data cols + 28 pad (col math: 124u+96 <= 1024)

_CACHE = {}


def _build_nc():
    import concourse.bass as bass
    import concourse.mybir as mybir
    import concourse.tile as tile
    from concourse import bacc

    F32 = mybir.dt.float32
    BF16 = mybir.dt.bfloat16

    nc = bacc.Bacc(
        "TRN2", target_bir_lowering=False, debug=False, num_devices=NCORES
    )
    W1 = nc.declare_dram_parameter("W1", [PP, O1, I], F32, isOutput=False)
    w2c = nc.declare_dram_parameter("w2c", [128, 16 * O1], F32, isOutput=False)
    bd3h = nc.declare_dram_parameter("bd3h", [128, 16 * 48], F32, isOutput=False)
    glfTh = nc.declare_dram_parameter("glfTh", [128, 128], F32, isOutput=False)
    b1r = nc.declare_dram_parameter("b1r", [128, 64], F32, isOutput=False)
    b2r = nc.declare_dram_parameter("b2r", [128, 16], F32, isOutput=False)
    b3rh = nc.declare_dram_parameter("b3rh", [1, PP * O3], F32, isOutput=False)
    out = nc.declare_dram_parameter("out", [B, O3, PP], F32, isOutput=True)

    with tile.TileContext(nc) as tc:
        with (
            tc.tile_pool(name="persist", bufs=1) as pp,
            tc.tile_pool(name="m2", bufs=4) as m2p,
            tc.tile_pool(name="rowp", bufs=2) as rowp,
            tc.tile_pool(name="pst", bufs=3, space="PSUM") as pstp,
            tc.tile_pool(name="psa", bufs=2, space="PSUM") as psap,
            tc.tile_pool(name="psb", bufs=1, space="PSUM") as psbp,
            tc.tile_pool(name="psy", bufs=2, space="PSUM") as psyp,
        ):
            # ---- W1 stream: 16 persistent bf16 tiles via gpsimd cast-DMA ----
            # bd3 loads via Pool cast-DMA first: lands as bf16 with no DVE
            # cast, and its descriptor-gen time lets the small HWDGE loads
            # win the DMA queue ahead of tile 0
            bd3 = pp.tile([128, 16 * 48], BF16, tag="bd3")
            nc.gpsimd.dma_start(out=bd3, in_=bd3h[:])
            w1tiles = []
            for t in range(NT):
                w1t = pp.tile([128, 4 * I], BF16, tag=f"w1t{t}", name=f"w1t{t}")
                w1tiles.append(w1t)
                w1src = (
                    W1[:]
                    .rearrange("p o i -> (p o) i")[512 * t : 512 * (t + 1), :]
                    .rearrange("(u q) i -> q u i", q=128)
                )
                w1dst = w1t[:].rearrange("q (u i) -> q u i", u=4)
                if t == NT - 1:
                    # split the last tile by u-rows so the final matmuls can
                    # start before the full tile lands
                    for u4 in range(4):
                        nc.gpsimd.dma_start(
                            out=w1dst[:, u4 : u4 + 1, :],
                            in_=w1src[:, u4 : u4 + 1, :],
                        )
                else:
                    nc.gpsimd.dma_start(out=w1dst, in_=w1src)

            # ---- dummy tile + zero-fills first (no load deps) ----
            dmy = pp.tile([128, 384], BF16, tag="dmy")
            nc.vector.memset(dmy, 0.0)
            bda = [
                pp.tile([128, 1024], BF16, tag=f"bda{i}", name=f"bda{i}")
                for i in range(4)
            ]
            nc.vector.memset(bda[0], 0.0)
            nc.scalar.memzero(bda[1][:])
            nc.scalar.memzero(bda[2][:])
            nc.scalar.memzero(bda[3][:])
            ones_f32 = pp.tile([1, B], F32, tag="ones32")
            nc.vector.memset(ones_f32, 1.0)
            ones_sb = pp.tile([1, B], BF16, tag="ones")
            nc.vector.tensor_copy(ones_sb, ones_f32)

            # ---- small inputs on HWDGE, critical first; casts split ----
            w2f = pp.tile([128, 16 * O1], F32, tag="w2f")
            nc.sync.dma_start(out=w2f, in_=w2c[:])
            b1f = pp.tile([128, 64], F32, tag="b1f")
            nc.scalar.dma_start(out=b1f, in_=b1r[:])
            b2f = pp.tile([128, 16], F32, tag="b2f")
            nc.scalar.dma_start(out=b2f, in_=b2r[:])
            glff = pp.tile([128, 128], F32, tag="glff")
            nc.scalar.dma_start(out=glff, in_=glfTh[:])
            b3f = pp.tile([1, PP * O3], F32, tag="b3f")
            nc.scalar.dma_start(out=b3f, in_=b3rh[:])

            outsb = pp.tile([B, 768], F32, tag="outsb")

            w2b = pp.tile([128, 16 * O1], BF16, tag="w2b")
            nc.vector.tensor_copy(w2b, w2f)
            b1b = pp.tile([128, 64], BF16, tag="b1b")
            nc.scalar.copy(b1b, b1f)
            b2b = pp.tile([128, 16], BF16, tag="b2b")
            nc.scalar.copy(b2b, b2f)

            # reserve the first pst pool slot (rotation parity matters to
            # the scheduler; removing this costs ~1us)
            dps = pstp.tile([128, 96], F32, tag="pst")

            # ================= steady-state per-tt pipeline =================
            def a32_and_scatter(tt):
                """A32 = W3@W2 for tt's 32 positions; scatter into BDA bands."""
                buf = bda[tt % 4]
                psA = psap.tile([B, 96], F32, tag="psa")
                for ch in range(2):
                    c16 = 2 * tt + ch
                    nc.tensor.matmul(
                        psA[:, 48 * ch : 48 * ch + 48],
                        lhsT=w2b[:, :].rearrange("q (c o) -> q c o", c=16)[
                            :, c16, :
                        ],
                        rhs=bd3[:, 48 * c16 : 48 * c16 + 48],
                        start=True,
                        stop=True,
                        skip_group_check=True,
                    )
                # BDA[32p4+o1, 124(4ch+u') + 32o3 + 16ch + 4u' + p4]
                #   = A32T[o1, 48ch + 16o3 + 4u' + p4]
                for p4 in range(4):
                    dst = buf[32 * p4 : 32 * p4 + 32, :].rearrange(
                        "q (cc a b r) -> q cc a b r", cc=2, a=4, r=32
                    )[:, :, :, 0:3, p4]
                    src = psA[:, :].rearrange(
                        "q (c b rr s) -> q c rr b s", c=2, b=3, s=4
                    )[:, :, :, :, p4]
                    # same-tile writes serialize across engines (sem hops);
                    # keep each tt's scatters on one engine
                    if tt % 2 == 0:
                        nc.vector.tensor_copy(dst, src)
                    else:
                        nc.scalar.copy(dst, src)

            def stage1(tt):
                """m2T [i-chunk slices, (o3,p)] for tt via operand swap.
                All 4 c-groups accumulate into one 1536B psum tile (one
                bank), drained by a single copy."""
                buf = bda[tt % 4]
                pst = pstp.tile([128, 4 * 96], F32, tag="pst")
                # one whole-tile start (zero stationary) so the u-major
                # accumulation below never trips the bank's pending-zero
                nc.tensor.matmul(
                    pst,
                    lhsT=dmy[:, 0:128],
                    rhs=dmy[:, 0:384],
                    start=True,
                    stop=False,
                    skip_group_check=True,
                )
                for u in range(NTT):
                    for c in range(4):
                        w1t = w1tiles[2 * tt + u // 4]
                        lhsT = w1t[:].rearrange("q (v i) -> q v i", v=4)[
                            :, u % 4, 128 * c : 128 * (c + 1)
                        ]
                        nc.tensor.matmul(
                            pst[:, 96 * c : 96 * (c + 1)],
                            lhsT=lhsT,
                            rhs=buf[:, BW * u : BW * u + 96],
                            start=False,
                            stop=(u == NTT - 1),
                            skip_group_check=True,
                        )
                m2t = m2p.tile([128, 4 * 96], BF16, tag="m2t")
                if tt % 2 == 0:
                    nc.scalar.copy(m2t, pst)
                else:
                    nc.vector.tensor_copy(m2t, pst)
                return m2t

            def bias_row(tt):
                """beff3^T row for tt: A32@b1 + W3@b2 (b3 added in stage2)."""
                buf = bda[tt % 4]
                pb = psbp.tile([1, 96], F32, tag="psb")
                for u in range(NTT):
                    g = 8 * tt + u
                    nc.tensor.matmul(
                        pb,
                        lhsT=b1b[:, g : g + 1],
                        rhs=buf[:, BW * u : BW * u + 96],
                        start=(u == 0),
                        stop=False,
                    )
                for ch in range(2):
                    c16 = 2 * tt + ch
                    nc.tensor.matmul(
                        pb[:, :].rearrange("o (b hh r) -> o b hh r", b=3, r=16)[
                            :, :, ch, :
                        ],
                        lhsT=b2b[:, c16 : c16 + 1],
                        rhs=bd3[:, 48 * c16 : 48 * c16 + 48],
                        start=False,
                        stop=(ch == 1),
                        skip_group_check=True,
                    )
                row = rowp.tile([1, 96], BF16, tag="row")
                nc.scalar.copy(row, pb)
                return row

            def stage2(tt, m2ts, row):
                # glfT matmuls first so the bias-row copy latency is hidden
                py = psyp.tile([B, 96], F32, tag="py")
                nc.tensor.matmul(
                    py,
                    lhsT=ones_sb,
                    rhs=b3r[0:1, 96 * tt : 96 * (tt + 1)],
                    start=True,
                    stop=False,
                )
                nc.tensor.matmul(
                    py, lhsT=ones_sb, rhs=row, start=False, stop=False
                )
                for c in range(4):
                    nc.tensor.matmul(
                        py,
                        lhsT=glfT[:, 32 * c : 32 * c + 32],
                        rhs=m2ts[:, 96 * c : 96 * (c + 1)],
                        start=False,
                        stop=(c == 3),
                    )
                dst = outsb[:, :].rearrange("q (b P) -> q b P", b=3)[
                    :, :, 32 * tt : 32 * (tt + 1)
                ]
                nc.vector.tensor_copy(
                    dst, py[:, :].rearrange("q (b r) -> q b r", b=3)
                )

            def a32_and_scatter_halves():
                """A32 for positions 224-255, scattered as two 16-position
                half-groups into bda[3]: band (h, u') at cols
                [512h + 124u', +48), within-band col = 16*o3 + 4*u' + p4."""
                buf = bda[3]
                psA = psap.tile([B, 96], F32, tag="psa")
                for ch in range(2):
                    c16 = 14 + ch
                    nc.tensor.matmul(
                        psA[:, 48 * ch : 48 * ch + 48],
                        lhsT=w2b[:, :].rearrange("q (c o) -> q c o", c=16)[
                            :, c16, :
                        ],
                        rhs=bd3[:, 48 * c16 : 48 * c16 + 48],
                        start=True,
                        stop=True,
                        skip_group_check=True,
                    )
                for p4 in range(4):
                    dst = buf[32 * p4 : 32 * p4 + 32, :].rearrange(
                        "q (hh a b r) -> q hh a b r", hh=2, a=4, r=16
                    )[:, :, :, 0:3, p4]
                    src = psA[:, :].rearrange(
                        "q (c b rr s) -> q c rr b s", c=2, b=3, s=4
                    )[:, :, :, :, p4]
                    nc.vector.tensor_copy(dst, src)

            def stage1_half(h):
                buf = bda[3]
                pst = pstp.tile([128, 4 * 96], F32, tag="pst")
                nc.tensor.matmul(
                    pst[:, 0 : 4 * 48],
                    lhsT=dmy[:, 0:128],
                    rhs=dmy[:, 0 : 4 * 48],
                    start=True,
                    stop=False,
                    skip_group_check=True,
                )
                for u in range(4):
                    for c in range(4):
                        lhsT = w1tiles[14 + h][:].rearrange(
                            "q (v i) -> q v i", v=4
                        )[:, u, 128 * c : 128 * (c + 1)]
                        nc.tensor.matmul(
                            pst[:, 48 * c : 48 * (c + 1)],
                            lhsT=lhsT,
                            rhs=buf[:, 512 * h + BW * u : 512 * h + BW * u + 48],
                            start=False,
                            stop=(u == 3),
                            skip_group_check=True,
                        )
                m2t = m2p.tile([128, 4 * 96], BF16, tag="m2t")
                if h == 0:
                    nc.scalar.copy(m2t[:, 0 : 4 * 48], pst[:, 0 : 4 * 48])
                else:
                    nc.vector.tensor_copy(m2t[:, 0 : 4 * 48], pst[:, 0 : 4 * 48])
                return m2t

            def bias_half(h):
                buf = bda[3]
                pb = psbp.tile([1, 96], F32, tag="psb")
                for u in range(4):
                    g = 56 + 4 * h + u
                    nc.tensor.matmul(
                        pb[:, 0:48],
                        lhsT=b1b[:, g : g + 1],
                        rhs=buf[:, 512 * h + BW * u : 512 * h + BW * u + 48],
                        start=(u == 0),
                        stop=False,
                        skip_group_check=True,
                    )
                c16 = 14 + h
                nc.tensor.matmul(
                    pb[:, 0:48],
                    lhsT=b2b[:, c16 : c16 + 1],
                    rhs=bd3[:, 48 * c16 : 48 * c16 + 48],
                    start=False,
                    stop=True,
                    skip_group_check=True,
                )
                row = rowp.tile([1, 96], BF16, tag="row")
                nc.scalar.copy(row[:, 0:48], pb[:, 0:48])
                return row

            def stage2_half(h, m2t, row):
                py = psyp.tile([B, 96], F32, tag="py")
                nc.tensor.matmul(
                    py[:, 0:48],
                    lhsT=ones_sb,
                    rhs=b3r[0:1, 672:768].rearrange(
                        "o (b hh r) -> o b hh r", b=3, hh=2
                    )[:, :, h, :],
                    start=True,
                    stop=False,
                    skip_group_check=True,
                )
                nc.tensor.matmul(
                    py[:, 0:48], lhsT=ones_sb, rhs=row[:, 0:48],
                    start=False, stop=False, skip_group_check=True,
                )
                for c in range(4):
                    nc.tensor.matmul(
                        py[:, 0:48],
                        lhsT=glfT[:, 32 * c : 32 * c + 32],
                        rhs=m2t[:, 48 * c : 48 * (c + 1)],
                        start=False,
                        stop=(c == 3),
                        skip_group_check=True,
                    )
                dst = outsb[:, :].rearrange("q (b P) -> q b P", b=3)[
                    :, :, 224 + 16 * h : 240 + 16 * h
                ]
                nc.vector.tensor_copy(
                    dst, py[:, 0:48].rearrange("q (b r) -> q b r", b=3)
                )
                nc.sync.dma_start(
                    out=out[:, :, 224 + 16 * h : 240 + 16 * h],
                    in_=dst,
                )

            a32_and_scatter(0)
            a32_and_scatter(1)
            glfT = pp.tile([128, 128], BF16, tag="glfT")
            nc.scalar.copy(glfT, glff)
            b3r = pp.tile([1, PP * O3], BF16, tag="b3r")
            nc.scalar.copy(b3r, b3f)
            # software pipeline: stage2 trails one period so its m2t/row
            # inputs are a full period old (no copy-latency waits)
            prev = None
            for tt in range(7):
                m2ts = stage1(tt)
                if prev is not None:
                    stage2(*prev)
                row = bias_row(tt)
                prev = (tt, m2ts, row)
                if tt + 2 < 7:
                    a32_and_scatter(tt + 2)
                elif tt + 2 == 7:
                    a32_and_scatter_halves()
            rowh0 = bias_half(0)
            m2h0 = stage1_half(0)
            stage2(*prev)
            nc.sync.dma_start(
                out=out[:, :, 0:224],
                in_=outsb[:, :].rearrange("q (b P) -> q b P", b=3)[:, :, 0:224],
            )
            rowh1 = bias_half(1)
            m2h1 = stage1_half(1)
            stage2_half(0, m2h0, rowh0)
            stage2_half(1, m2h1, rowh1)

    nc.compile()
    return nc


def _get_nc():
    if "nc" not in _CACHE:
        _CACHE["nc"] = _build_nc()
    return _CACHE["nc"]


def _layout_prep(W2s, W3s, b1s, b2s, b3s, glf):
    """Pure layout reindexing (no arithmetic) of the small per-core tensors."""
    w2c = np.ascontiguousarray(
        W2s.reshape(PP * O2, O1).reshape(16, 128, O1).transpose(1, 0, 2)
    ).reshape(128, 16 * O1)
    bd3h = np.zeros((128, 16 * 48), dtype=np.float32)
    for p16 in range(16):
        # bd3h[8*p16+o2, 48g+16o3+p16] = W3s[16g+p16, o3, o2]
        blk = W3s[p16::16, :, :]            # [16(g), 3(o3), 8(o2)]
        bd3h[8 * p16 : 8 * p16 + 8, :].reshape(8, 16, 3, 16)[
            :, :, :, p16
        ] = blk.transpose(2, 0, 1)
    glfTh = np.ascontiguousarray(
        glf.T.reshape(4, 128, B).transpose(1, 0, 2)
    ).reshape(128, 128)
    b1rh = np.ascontiguousarray(b1s.reshape(-1).reshape(64, 128).T)
    b2rh = np.ascontiguousarray(b2s.reshape(-1).reshape(16, 128).T)
    b3rh = np.ascontiguousarray(
        b3s.reshape(8, 32, O3).transpose(0, 2, 1)
    ).reshape(1, PP * O3)
    return w2c, bd3h, glfTh, b1rh, b2rh, b3rh


def _make_in_maps(inputs):
    glf = np.ascontiguousarray(
        np.asarray(inputs["glf"], dtype=np.float32).reshape(B, I)
    )
    ins = {k: np.asarray(inputs[k], dtype=np.float32) for k in
           ("W1", "b1", "W2", "b2", "W3", "b3")}
    in_maps = []
    for c in range(NCORES):
        sl = slice(c * PP, (c + 1) * PP)
        w2c, bd3h, glfTh, b1rh, b2rh, b3rh = _layout_prep(
            ins["W2"][sl], ins["W3"][sl], ins["b1"][sl], ins["b2"][sl],
            ins["b3"][sl], glf,
        )
        in_maps.append(
            {
                "W1": np.ascontiguousarray(ins["W1"][sl]),
                "w2c": w2c,
                "bd3h": bd3h,
                "glfTh": glfTh,
                "b1r": b1rh,
                "b2r": b2rh,
                "b3rh": b3rh,
            }
        )
    return in_maps


def run(inputs, trace=False):
    """Run on the 8 NeuronCores; returns (out_full, BassKernelResults)."""
    from concourse.bass_utils import run_bass_kernel_spmd

    nc = _get_nc()
    res = run_bass_kernel_spmd(
        nc, _make_in_maps(inputs), list(range(NCORES)), trace=trace
    )
    out_full = np.empty((B, O3, P_FULL), dtype=np.float32)
    for c in range(NCORES):
        out_full[:, :, c * PP : (c + 1) * PP] = res.results[c]["out"]
    return out_full, res


def kernel(**inputs):
    out, _ = run(inputs, trace=False)
    return out
